# revision 5
# baseline (speedup 1.0000x reference)
"""Bass/Tile kernel builder for distributed causal MHA with RoPE on 8 NeuronCores.

Sharding: head-pair per core (16 heads / 8 cores = 2 heads each), both batches
on every core.  After attention, one 8-core AllToAll redistributes the per-head
context so core c assembles the full context for (batch c//4, seq-quarter c%4)
and applies the output projection locally.  Host concatenates the 8 quarters.

All matmuls run as float32r (FP22-truncated fp32) at full PE rate.
"""

import sys

sys.path.insert(0, "/opt/trn_rl_repo")

import numpy as np
import concourse.bass as bass
import concourse.mybir as mybir
import concourse.tile as tile
from concourse import bacc
from concourse.masks import make_identity

F32 = mybir.dt.float32
F32R = mybir.dt.float32r

D_MODEL = 1024
NUM_HEADS = 16
DHEAD = 64
THETA = 10000.0
N_CORES = 8
B = 2


def r(ap):
    """bitcast an fp32 AP to float32r for matmul operands."""
    return ap.bitcast(F32R)


def build_nc(S):
    """Build the SPMD Bass program (identical on all 8 cores)."""
    assert S % 512 == 0
    SQ = S // 4            # seq quarter each core outputs
    NJ = S // 512          # number of 512-wide sq chunks
    NK = S // 128          # number of 128-tall sk tiles
    CW = min(512, SQ)      # chunk width in the Wo phase
    NC2 = SQ // CW         # chunks per quarter
    NST = SQ // 128        # 128-row out tiles per quarter

    nc = bacc.Bacc("TRN2", target_bir_lowering=False, debug=False,
                   num_devices=N_CORES)

    # ---- I/O ----
    xt = nc.dram_tensor("xt", [B, D_MODEL, S], F32, kind="ExternalInput")
    wq = nc.dram_tensor("wq", [D_MODEL, 128], F32, kind="ExternalInput")
    wk = nc.dram_tensor("wk", [D_MODEL, 128], F32, kind="ExternalInput")
    wv = nc.dram_tensor("wv", [D_MODEL, 128], F32, kind="ExternalInput")
    wo = nc.dram_tensor("wo", [D_MODEL, D_MODEL], F32, kind="ExternalInput")
    cosm = nc.dram_tensor("cosm", [128, S], F32, kind="ExternalInput")
    sinm = nc.dram_tensor("sinm", [128, S], F32, kind="ExternalInput")
    sel2 = nc.dram_tensor("sel2", [2, 128], F32, kind="ExternalInput")
    out = nc.dram_tensor("out", [SQ, D_MODEL], F32, kind="ExternalOutput")

    with tile.TileContext(nc) as tc:
        with (
            tc.tile_pool(name="persist", bufs=1) as pp,
            tc.tile_pool(name="dram", bufs=1, space="DRAM") as dram,
        ):
            qp = tc.alloc_tile_pool(name="qkv", bufs=1)
            # long-lived sbuf tensors (released after attention)
            qt = [qp.tile([128, S], F32R, name=f"qt{b}") for b in range(B)]
            kt = [qp.tile([128, S], F32R, name=f"kt{b}") for b in range(B)]
            vsb = [[qp.tile([128, 130], F32R, name=f"v{b}_{st}")
                    for st in range(NK)] for b in range(B)]
            sel2_sb = pp.tile([2, 128], F32R, name="sel2_sb")
            nc.sync.dma_start(sel2_sb[:], r(sel2[:]))
            onesc = pp.tile([128, 2], F32, name="onesc")
            nc.vector.memset(onesc[:], 1.0)

            # ---------------- Phase 1: projections ----------------
            with (
                tc.tile_pool(name="wts", bufs=1) as wp,
                tc.tile_pool(name="xch", bufs=2) as xp,
                tc.tile_pool(name="p1ps", bufs=2, space="PSUM") as ps1,
                tc.tile_pool(name="vps", bufs=2, space="PSUM") as psv,
            ):
                wq_sb = wp.tile([128, 8, 128], F32R, name="wq_sb")
                wk_sb = wp.tile([128, 8, 128], F32R, name="wk_sb")
                wv_sb = wp.tile([128, 8, 128], F32R, name="wv_sb")
                ident = wp.tile([128, 128], F32, name="ident")
                make_identity(nc, ident[:])
                for kk in range(8):
                    nc.sync.dma_start(wq_sb[:, kk, :], r(wq[128 * kk:128 * kk + 128, :]))
                    nc.sync.dma_start(wk_sb[:, kk, :], r(wk[128 * kk:128 * kk + 128, :]))
                    nc.sync.dma_start(wv_sb[:, kk, :], r(wv[128 * kk:128 * kk + 128, :]))

                for b in range(B):
                    for sc in range(NJ):
                        s0 = 512 * sc
                        xch = xp.tile([128, 8, 512], F32R, name="xch", tag="xch")
                        for kk in range(8):
                            nc.sync.dma_start(
                                xch[:, kk, :],
                                r(xt[b, 128 * kk:128 * kk + 128, s0:s0 + 512]))
                        q_ps = ps1.tile([128, 512], F32, name="q_ps", tag="q")
                        k_ps = ps1.tile([128, 512], F32, name="k_ps", tag="k")
                        vt_ps = ps1.tile([128, 512], F32, name="vt_ps", tag="vt")
                        for kk in range(8):
                            nc.tensor.matmul(q_ps[:], r(wq_sb[:, kk, :]),
                                             r(xch[:, kk, :]),
                                             start=(kk == 0), stop=(kk == 7))
                        for kk in range(8):
                            nc.tensor.matmul(k_ps[:], r(wk_sb[:, kk, :]),
                                             r(xch[:, kk, :]),
                                             start=(kk == 0), stop=(kk == 7))
                        for kk in range(8):
                            nc.tensor.matmul(vt_ps[:], r(wv_sb[:, kk, :]),
                                             r(xch[:, kk, :]),
                                             start=(kk == 0), stop=(kk == 7))
                        nc.vector.tensor_copy(qt[b][:, s0:s0 + 512], q_ps[:])
                        nc.vector.tensor_copy(kt[b][:, s0:s0 + 512], k_ps[:])
                        vt_sb = xp.tile([128, 512], F32, name="vt_sb", tag="vtsb")
                        nc.vector.tensor_copy(vt_sb[:], vt_ps[:])
                        # transpose [m, s] -> [s, m] per 128-block via PE
                        for st in range(4):
                            v_ps = psv.tile([128, 128], F32, name="v_ps", tag="v")
                            nc.tensor.transpose(
                                v_ps[:], vt_sb[:, 128 * st:128 * st + 128],
                                ident[:])
                            vt = vsb[b][4 * sc + st]
                            # layout [V_h0 | 1 | V_h1 | 1]: ones at cols 64, 129
                            vt3 = vt[:].rearrange("p (a b) -> p a b", a=2)
                            nc.vector.tensor_copy(
                                vt3[:, :, 64:65],
                                onesc[:].rearrange("p (a b) -> p a b", a=2))
                            nc.vector.tensor_copy(
                                vt3[:, :, 0:64],
                                v_ps[:].rearrange("p (a b) -> p a b", a=2))

            # ---------------- Phase 2: rope ----------------
            with (
                tc.tile_pool(name="tables", bufs=1) as tabp,
                tc.tile_pool(name="ropetmp", bufs=2) as rp,
            ):
                cos_sb = tabp.tile([128, S], F32, name="cos_sb")
                sin_sb = tabp.tile([128, S], F32, name="sin_sb")
                nc.sync.dma_start(cos_sb[:], cosm[:])
                nc.sync.dma_start(sin_sb[:], sinm[:])
                HW_ = 2048 if S >= 2048 else S
                for b in range(B):
                    for ten in (qt[b], kt[b]):
                        for c0 in range(0, S, HW_):
                            sl = slice(c0, c0 + HW_)
                            t1 = rp.tile([128, HW_], F32, name="t1", tag="t1")
                            t2 = rp.tile([128, HW_], F32, name="t2", tag="t2")
                            t2s = rp.tile([128, HW_], F32, name="t2s", tag="t2s")
                            nc.vector.tensor_mul(t1[:], ten[:, sl], cos_sb[:, sl])
                            nc.vector.tensor_mul(t2[:], ten[:, sl], sin_sb[:, sl])
                            for blk in range(4):
                                src = 32 * (blk ^ 1)
                                nc.sync.dma_start(t2s[32 * blk:32 * blk + 32, :],
                                                  t2[src:src + 32, :])
                            nc.vector.tensor_add(ten[:, sl], t1[:], t2s[:])

            # ---------------- Phase 3: attention ----------------
            ib = dram.tile([8, 130, SQ], F32, name="ib")
            ob = dram.tile([8, 130, SQ], F32, name="ob")
            with (
                tc.tile_pool(name="scps", bufs=2, space="PSUM") as scp,
                tc.tile_pool(name="avps", bufs=1, space="PSUM") as avp,
                tc.tile_pool(name="ptp", bufs=3) as ptp,
                tc.tile_pool(name="cxp", bufs=4) as cxp,
            ):
                for b in range(B):
                    for j in range(NJ):
                        nk = min(4 * j + 4, NK)
                        q0 = 512 * j
                        nslot = 2 * nk
                        ngroup = (nslot + 2) // 3
                        sc_t = [scp.tile([128, 1536], F32, name="sc_t", tag="sc")
                                for _ in range(ngroup)]
                        pt_t = [ptp.tile([128, 1536], F32R, name="pt_t", tag="pt")
                                for _ in range(ngroup)]

                        def slot_ap(tiles, s):
                            return tiles[s // 3][:, 512 * (s % 3):512 * (s % 3) + 512]

                        # scores + exp
                        for k in range(nk):
                            for h in range(2):
                                s = 2 * k + h
                                hb = 64 * h
                                nc.tensor.matmul(
                                    slot_ap(sc_t, s),
                                    r(kt[b][hb:hb + 64, 128 * k:128 * k + 128]),
                                    r(qt[b][hb:hb + 64, q0:q0 + 512]),
                                    start=True, stop=True)
                            # when a group of 3 fills (or last slot), exp it
                        for g in range(ngroup):
                            w = min(1536, (nslot - 3 * g) * 512)
                            nc.scalar.activation(pt_t[g][:, 0:w], sc_t[g][:, 0:w],
                                                 mybir.ActivationFunctionType.Exp,
                                                 scale=0.125)
                        # causal mask on band tiles (k in [4j, 4j+3])
                        for k in range(max(0, 4 * j), nk):
                            base = 512 * j - 128 * k
                            for h in range(2):
                                s = 2 * k + h
                                ap = slot_ap(pt_t, s)
                                nc.gpsimd.affine_select(
                                    ap, ap, pattern=[[1, 512]],
                                    compare_op=mybir.AluOpType.is_ge,
                                    fill=0.0, base=base, channel_multiplier=-1)
                        # AV: interleave both heads' accumulation chains so pt
                        # groups retire in slot order (lets the pt pool recycle)
                        av = [avp.tile([65, 512], F32, name=f"av{h}", tag=f"av{h}")
                              for h in range(2)]
                        for k in range(nk):
                            for h in range(2):
                                nc.tensor.matmul(
                                    av[h][:], r(vsb[b][k][:, 65 * h:65 * h + 65]),
                                    r(slot_ap(pt_t, 2 * k + h)),
                                    start=(k == 0), stop=(k == nk - 1))
                        # drain ctx+denom to SBUF (partition-aligned), recip the
                        # denom, then DMA straight into the A2A input bounce
                        for h in range(2):
                            cx = cxp.tile([65, 512], F32, name="cx", tag="cx")
                            nc.vector.tensor_copy(cx[:], av[h][:])
                            nc.vector.reciprocal(cx[64:65, :], cx[64:65, :])
                            # split the 512-chunk by seq-quarter boundaries
                            c0 = q0
                            while c0 < q0 + 512:
                                g2 = c0 // SQ
                                w = min(SQ * (g2 + 1), q0 + 512) - c0
                                j2 = b * 4 + g2
                                lo, li = c0 - SQ * g2, c0 - q0
                                nc.sync.dma_start(
                                    ib[j2, 0 + 64 * h:64 * h + 64, lo:lo + w],
                                    cx[0:64, li:li + w])
                                nc.sync.dma_start(
                                    ib[j2, 128 + h:129 + h, lo:lo + w],
                                    cx[64:65, li:li + w])
                                c0 += w

            qp.release()

            # ---------------- Phase 4: A2A + output projection ----------------
            nc.gpsimd.collective_compute(
                "AllToAll", mybir.AluOpType.bypass,
                replica_groups=[list(range(8))],
                ins=[ib.opt()], outs=[ob.opt()])

            with (
                tc.tile_pool(name="wophase", bufs=1) as wop,
                tc.tile_pool(name="ctxsp", bufs=1) as csp,
                tc.tile_pool(name="wops", bufs=2, space="PSUM") as wops,
                tc.tile_pool(name="bcps", bufs=2, space="PSUM") as bcps,
                tc.tile_pool(name="osbp", bufs=3) as osbp,
            ):
                wo_sb = wop.tile([128, 8, D_MODEL], F32R, name="wo_sb")
                for t in range(8):
                    nc.sync.dma_start(wo_sb[:, t, :], r(wo[128 * t:128 * t + 128, :]))
                ctxs = []
                for t in range(8):
                    ctxf = wop.tile([128, SQ], F32, name=f"ctxf{t}")
                    rq = wop.tile([2, SQ], F32R, name=f"rq{t}")
                    nc.sync.dma_start(ctxf[:], ob[t, 0:128, :])
                    nc.sync.dma_start(rq[:], r(ob[t, 128:130, :]))
                    row = []
                    for c2 in range(NC2):
                        cl = slice(CW * c2, CW * (c2 + 1))
                        bc = bcps.tile([128, CW], F32, name="bc", tag="bc")
                        nc.tensor.matmul(bc[:], r(sel2_sb[:]), r(rq[:, cl]),
                                         start=True, stop=True)
                        cst = csp.tile([128, CW], F32R, name=f"ctxs{t}_{c2}")
                        nc.vector.tensor_mul(cst[:], ctxf[:, cl], bc[:])
                        row.append(cst)
                    ctxs.append(row)
                for st in range(NST):
                    for m2 in range(2):
                        wo_ps = wops.tile([128, 512], F32, name="wo_ps", tag="wo")
                        for t in range(8):
                            cst = ctxs[t][(128 * st) // CW]
                            coff = (128 * st) % CW
                            nc.tensor.matmul(
                                wo_ps[:], r(cst[:, coff:coff + 128]),
                                r(wo_sb[:, t, 512 * m2:512 * m2 + 512]),
                                start=(t == 0), stop=(t == 7))
                        osb = osbp.tile([128, 512], F32, name="osb", tag="osb")
                        nc.vector.tensor_copy(osb[:], wo_ps[:])
                        nc.sync.dma_start(
                            out[128 * st:128 * st + 128, 512 * m2:512 * m2 + 512],
                            osb[:])

    nc.compile()
    return nc


# ---------------------------------------------------------------------------
# Host-side sharding / assembly
# ---------------------------------------------------------------------------

def _rope_tables(token_positions, S):
    half = DHEAD // 2
    inv_freq = THETA ** (-2.0 * np.arange(half, dtype=np.float32) / DHEAD)
    angles = np.arange(4096, dtype=np.float32)[:, None] * inv_freq[None, :]
    cos_c, sin_c = np.cos(angles), np.sin(angles)
    pos = np.asarray(token_positions).astype(np.int64)
    cosT = cos_c[pos].T.astype(np.float32)   # [32, S]
    sinT = sin_c[pos].T.astype(np.float32)
    cosm = np.concatenate([cosT, cosT, cosT, cosT], 0)
    sinm = np.concatenate([sinT, -sinT, sinT, -sinT], 0)
    return np.ascontiguousarray(cosm), np.ascontiguousarray(sinm)


def prepare_in_maps(in_features, token_positions, Wq, Wk, Wv, Wo):
    Bb, S, D = in_features.shape
    xt = np.ascontiguousarray(in_features.transpose(0, 2, 1)).astype(np.float32)
    cosm, sinm = _rope_tables(token_positions, S)
    sel2 = np.zeros((2, 128), np.float32)
    sel2[0, :64] = 1.0
    sel2[1, 64:] = 1.0
    perm = np.concatenate([np.arange(0, 64, 2), np.arange(1, 64, 2)])
    woT = np.ascontiguousarray(Wo.T).astype(np.float32)
    in_maps = []
    for c in range(N_CORES):
        h0, h1 = 2 * c, 2 * c + 1
        blocks_qk = []
        for W in (Wq, Wk):
            cols = []
            for h in (h0, h1):
                blk = W[64 * h:64 * h + 64, :][perm, :]   # [64, D] permuted
                cols.append(blk.T)                         # [D, 64]
            blocks_qk.append(np.ascontiguousarray(
                np.concatenate(cols, axis=1)).astype(np.float32))
        wv_c = np.ascontiguousarray(np.concatenate(
            [Wv[64 * h:64 * h + 64, :].T for h in (h0, h1)], axis=1)).astype(np.float32)
        in_maps.append({
            "xt": xt, "wq": blocks_qk[0], "wk": blocks_qk[1], "wv": wv_c,
            "wo": woT, "cosm": cosm, "sinm": sinm, "sel2": sel2,
        })
    return in_maps


def assemble(results, S):
    SQ = S // 4
    out = np.zeros((B, S, D_MODEL), np.float32)
    for c in range(N_CORES):
        b, g = c // 4, c % 4
        out[b, SQ * g:SQ * (g + 1), :] = results[c]["out"]
    return out

from concourse.bass_utils import run_bass_kernel_spmd

_S = 4096
_NC = None


def _get_nc():
    global _NC
    if _NC is None:
        _NC = build_nc(_S)
    return _NC


def kernel(in_features, token_positions, Wq, Wk, Wv, Wo):
    x = np.asarray(in_features, dtype=np.float32)
    pos = np.asarray(token_positions)
    Wq = np.asarray(Wq, dtype=np.float32)
    Wk = np.asarray(Wk, dtype=np.float32)
    Wv = np.asarray(Wv, dtype=np.float32)
    Wo = np.asarray(Wo, dtype=np.float32)
    nc = _get_nc()
    in_maps = prepare_in_maps(x, pos, Wq, Wk, Wv, Wo)
    res = run_bass_kernel_spmd(nc, in_maps, list(range(N_CORES)))
    return assemble(res.results, _S)
